# revision 10
# baseline (speedup 1.0000x reference)
"""Trainium2 Bass kernel for nn_AnomalyGenerator (8-core data parallel).

The reference duplicates the batch (16 -> 32 samples), builds per-sample
Perlin threshold masks (64x64 canvas, bilinear-resized to 56x56), and
computes perturbed = input + noise * noise_mask over [32,768,56,56].

Key algebraic reduction: the Perlin gather + fade interpolation + bilinear
resize is linear in the per-sample gradient tables, so
    pn56^T = RBxT[ey]^T @ (gx^T @ LAxT[ex]) + RByT[ey]^T @ (gy^T @ LAyT[ex])
with 6-entry compile-time tables (scales are 2^e, e in 0..5), gx=cos(angles),
gy=sin(angles).  Four [33,*] matmuls per sample on the PE.

Sharding: core i owns input rows {2i, 2i+1} and duplicated samples
(slots) [2i, 2i+16, 2i+1, 2i+17] -- input rows are read once per core.
"""

import math
import ml_dtypes
import numpy as np

import concourse.bass as bass
import concourse.bacc as bacc
import concourse.mybir as mybir
import concourse.tile as tile
from concourse.bass_utils import run_bass_kernel_spmd

F32 = mybir.dt.float32
BF16 = mybir.dt.bfloat16
ALU = mybir.AluOpType
ACT = mybir.ActivationFunctionType

N_CORES = 8
B = 16            # original batch
C = 768
H = W = 56
HW = H * W        # 3136
CT = 3            # channel tiles of 256 (2 channels per partition, bf16)
CW = 2 * HW       # free width of a big tile: 2 channel rows of 3136
SQRT2 = 1.4142135623730951
BIG = 1.0e9

TRACE = False     # test.py sets True to collect HW exec time
_PROGRAM_CACHE = {}


def _fade(t):
    return t * t * t * (t * (t * 6.0 - 15.0) + 10.0)


def _build_tables():
    """ctab [33, 6*4*56] and cmisc [56, 128] compile-time constants."""
    # bilinear resize matrix L [56,64], align_corners=False
    L = np.zeros((56, 64), np.float64)
    sy = np.clip((np.arange(56) + 0.5) * (64.0 / 56.0) - 0.5, 0.0, 63.0)
    y0 = np.floor(sy).astype(int)
    y1 = np.minimum(y0 + 1, 63)
    wy = sy - y0
    for h in range(56):
        L[h, y0[h]] += 1.0 - wy[h]
        L[h, y1[h]] += wy[h]

    ctab = np.zeros((33, 6 * 4 * 56), np.float64)
    for e in range(6):
        s = 2 ** e
        i = np.arange(64)
        ix = i * s
        c0 = ix // 64
        fu = (ix % 64).astype(np.float64) / 64.0
        tu = _fade(fu)
        U0 = np.zeros((64, 33))
        U1 = np.zeros((64, 33))
        U0[np.arange(64), c0] = 1.0
        U1[np.arange(64), c0 + 1] = 1.0
        Ax = (((1 - tu) * fu)[:, None] * U0) + ((tu * (fu - 1))[:, None] * U1)
        Ay = ((1 - tu)[:, None] * U0) + (tu[:, None] * U1)
        Bx = Ay                                   # v-factor for x-gradient
        By = Ax                                   # v-factor for y-gradient
        base = e * 224
        ctab[:, base + 0:base + 56] = (SQRT2 * (L @ Ax)).T     # LAxT
        ctab[:, base + 56:base + 112] = (SQRT2 * (L @ Ay)).T   # LAyT
        ctab[:, base + 112:base + 168] = (L @ Bx).T            # RBxT
        ctab[:, base + 168:base + 224] = (L @ By).T            # RByT

    cmisc = np.zeros((56, 256), np.float64)
    cmisc[:, 0:6] = np.arange(6)[None, :]         # iota row 0..5
    cmisc[:, 6:62] = np.eye(56)                   # identity for PE transpose
    cmisc[:, 62:190] = 1.0                        # ones block (128 wide)
    cmisc[:, 190] = math.pi                       # pi (activation bias)
    return ctab.astype(np.float32), cmisc.astype(np.float32)


def _build_program():
    """Build + compile the per-core Bass program (same NEFF on all 8 cores)."""
    ctab_np, cmisc_np = _build_tables()

    nc = bacc.Bacc("TRN2", target_bir_lowering=False, debug=False,
                   num_devices=N_CORES)

    inp = nc.dram_tensor("inp", [2, CT, 128, CW], BF16, kind="ExternalInput")
    noz = nc.dram_tensor("noz", [4, CT, 128, CW], BF16, kind="ExternalInput")
    msk = nc.dram_tensor("msk", [2, H, W], F32, kind="ExternalInput")
    ang = nc.dram_tensor("ang", [4, 33, 33], F32, kind="ExternalInput")
    scl = nc.dram_tensor("scl", [1, 16], F32, kind="ExternalInput")

    pert = nc.dram_tensor("pert", [4, CT, 128, CW], BF16, kind="ExternalOutput")
    mout = nc.dram_tensor("mout", [4, H, W], F32, kind="ExternalOutput")
    lout = nc.dram_tensor("lout", [1, 4], F32, kind="ExternalOutput")

    ctab_h = nc.inline_tensor(ctab_np, "ctab")
    cmisc_h = nc.inline_tensor(cmisc_np, "cmisc")

    with tile.TileContext(nc) as tc:
        with (
            tc.tile_pool(name="const", bufs=1) as constp,
            tc.tile_pool(name="small", bufs=1) as smallp,
            tc.tile_pool(name="sel", bufs=2) as selp,
            tc.tile_pool(name="slot", bufs=2) as slotp,
            tc.tile_pool(name="nmb", bufs=1) as nmbp,
            tc.tile_pool(name="bigin", bufs=5) as binp,
            tc.tile_pool(name="bignz", bufs=4) as bnzp,
            tc.tile_pool(name="ps", bufs=1, space="PSUM") as psp,
        ):
            # ---- constants into SBUF ----
            tabs = constp.tile([33, 6 * 4 * 56], F32, tag="tabs")
            misc = constp.tile([56, 256], F32, tag="misc")
            nc.sync.dma_start(tabs[:, :], ctab_h.ap())
            nc.sync.dma_start(misc[:, :], cmisc_h.ap())
            iota33 = misc[0:33, 0:6]
            eye56 = misc[0:56, 6:62]
            ones_r33 = misc[0:1, 62:95]
            ones_r56 = misc[0:1, 62:118]
            ones_r128 = misc[0:1, 62:190]
            ones_c56 = misc[0:56, 62:63]
            pi33 = misc[0:33, 190:191]

            # ---- small per-core inputs ----
            sc = smallp.tile([1, 16], F32, tag="sc")
            nc.sync.dma_start(sc[:, :], scl.ap())
            gang = smallp.tile([33, 132], F32, tag="gang")
            for s in range(4):
                nc.sync.dma_start(gang[:, 33 * s:33 * (s + 1)], ang.ap()[s])
            msk_sb = smallp.tile([56, 112], F32, tag="msk")
            for j in range(2):
                nc.sync.dma_start(msk_sb[:, 56 * j:56 * (j + 1)], msk.ap()[j])

            # input loads first on the SWDGE (gpsimd) ring -- nothing with a
            # compute-wait may precede them there, so they stream from t=0
            its = []
            for j in range(2):
                for ct in range(CT):
                    it = binp.tile([128, CW], BF16, tag="in",
                                   name=f"it_{j}_{ct}")
                    nc.gpsimd.dma_start(it[:, :], inp.ap()[j, ct])
                    its.append(it)

            # ---- gradients: gx = cos(angles), gy = sin(angles)  [33,132] ----
            gy = smallp.tile([33, 132], F32, tag="gy")
            # sin(x) = sin(pi - x), arg in [-pi, pi]
            nc.scalar.activation(gy[:, :], gang[:, :], ACT.Sin,
                                 bias=pi33, scale=-1.0)
            gx = smallp.tile([33, 132], F32, tag="gx")
            u = smallp.tile([33, 132], F32, tag="u")
            nc.scalar.activation(u[:, :], gang[:, :], ACT.Sin, scale=0.5)
            nc.scalar.activation(u[:, :], u[:, :], ACT.Square)
            # cos(x) = 1 - 2 sin^2(x/2)
            nc.vector.tensor_scalar(gx[:, :], u[:, :], -2.0, 1.0,
                                    ALU.mult, ALU.add)

            # ---- broadcast per-slot scalars to 33 partitions ----
            prep_ps = psp.tile([33, 16], F32, tag="prep")
            nc.tensor.matmul(prep_ps[:, :], ones_r33, sc[:, :],
                             start=True, stop=True)
            scb33 = smallp.tile([33, 16], F32, tag="scb33")
            nc.scalar.copy(scb33[:, :], prep_ps[:, :])

            # one-hot scale selectors wxb/wyb [33, 24] (6 per slot)
            wxb = smallp.tile([33, 24], F32, tag="wxb")
            wyb = smallp.tile([33, 24], F32, tag="wyb")
            for s in range(4):
                nc.vector.tensor_scalar(wxb[:, 6 * s:6 * (s + 1)], iota33,
                                        scb33[:, s:s + 1], None, ALU.is_equal)
                nc.vector.tensor_scalar(wyb[:, 6 * s:6 * (s + 1)], iota33,
                                        scb33[:, 4 + s:5 + s], None,
                                        ALU.is_equal)

            # ---- per-slot threshold + (1-label) rows, broadcast to 56 ----
            sc8 = smallp.tile([1, 8], F32, tag="sc8")
            # T_s = 0.5 + BIG*(chance>0.5)
            nc.vector.tensor_scalar(sc8[:, 0:4], sc[0:1, 8:12], 0.5, None,
                                    ALU.is_gt)
            nc.vector.tensor_scalar(sc8[:, 0:4], sc8[:, 0:4], BIG, 0.5,
                                    ALU.mult, ALU.add)
            # (1 - label)
            nc.vector.tensor_scalar(sc8[:, 4:8], sc[0:1, 12:16], -1.0, 1.0,
                                    ALU.mult, ALU.add)
            prep2_ps = psp.tile([56, 8], F32, tag="prep")
            nc.tensor.matmul(prep2_ps[:, :], ones_r56, sc8[:, :],
                             start=True, stop=True)
            thb56 = smallp.tile([56, 8], F32, tag="thb56")
            nc.scalar.copy(thb56[:, :], prep2_ps[:, :])

            # (1 - mask) for the two input rows
            um_sb = smallp.tile([56, 112], F32, tag="um")
            nc.vector.tensor_scalar(um_sb[:, :], msk_sb[:, :], -1.0, 1.0,
                                    ALU.mult, ALU.add)

            sums = smallp.tile([1, 4], F32, tag="sums")
            nmb = [nmbp.tile([128, CW], BF16, tag=f"nmb{s}", name=f"nmb{s}")
                   for s in range(4)]

            # ---- per-slot perlin mask chain ----
            for s in range(4):
                j = s // 2  # input row index for this slot
                selx = selp.tile([33, 224], F32, tag="sel")
                # select tables for this slot's (ex, ey) via one-hot weights
                nc.vector.tensor_scalar(
                    selx[:, 0:112], tabs[:, 0:112],
                    wxb[:, 6 * s:6 * s + 1], None, ALU.mult)
                nc.vector.tensor_scalar(
                    selx[:, 112:224], tabs[:, 112:224],
                    wyb[:, 6 * s:6 * s + 1], None, ALU.mult)
                for e in range(1, 6):
                    nc.vector.scalar_tensor_tensor(
                        selx[:, 0:112], tabs[:, 224 * e:224 * e + 112],
                        wxb[:, 6 * s + e:6 * s + e + 1], selx[:, 0:112],
                        ALU.mult, ALU.add)
                    nc.vector.scalar_tensor_tensor(
                        selx[:, 112:224], tabs[:, 224 * e + 112:224 * e + 224],
                        wyb[:, 6 * s + e:6 * s + e + 1], selx[:, 112:224],
                        ALU.mult, ALU.add)

                gx_s = gx[:, 33 * s:33 * (s + 1)]
                gy_s = gy[:, 33 * s:33 * (s + 1)]
                p1_ps = psp.tile([33, 112], F32, tag="p1")
                nc.tensor.matmul(p1_ps[:, 0:56], gx_s, selx[:, 0:56],
                                 start=True, stop=True)
                nc.tensor.matmul(p1_ps[:, 56:112], gy_s, selx[:, 56:112],
                                 start=True, stop=True)
                p1s = slotp.tile([33, 112], F32, tag="p1s")
                nc.scalar.copy(p1s[:, :], p1_ps[:, :])

                pnT_ps = psp.tile([56, 56], F32, tag="pnT")
                nc.tensor.matmul(pnT_ps[:, :], selx[:, 112:168], p1s[:, 0:56],
                                 start=True, stop=False)
                nc.tensor.matmul(pnT_ps[:, :], selx[:, 168:224],
                                 p1s[:, 56:112], start=False, stop=True)
                pnT_sb = slotp.tile([56, 56], F32, tag="pnT_sb")
                nc.scalar.copy(pnT_sb[:, :], pnT_ps[:, :])

                pn_ps = psp.tile([56, 56], F32, tag="pn")
                nc.tensor.transpose(pn_ps[:, :], pnT_sb[:, :], eye56)

                thr = slotp.tile([56, 56], F32, tag="thr")
                nc.vector.tensor_scalar(thr[:, :], pn_ps[:, :],
                                        thb56[:, s:s + 1], None, ALU.is_gt)
                nm = slotp.tile([56, 56], F32, tag="nm")
                nc.vector.scalar_tensor_tensor(
                    nm[:, :], thr[:, :], thb56[:, 4 + s:5 + s],
                    um_sb[:, 56 * j:56 * (j + 1)], ALU.mult, ALU.mult)

                mob = slotp.tile([56, 56], F32, tag="mob")
                nc.vector.tensor_tensor(mob[:, :],
                                        msk_sb[:, 56 * j:56 * (j + 1)],
                                        nm[:, :], ALU.max)
                nc.vector.tensor_scalar(mob[:, :], mob[:, :], 0.0, None,
                                        ALU.is_gt)
                nc.gpsimd.dma_start(mout.ap()[s], mob[:, :])

                cs_ps = psp.tile([1, 56], F32, tag="cs")
                nc.tensor.matmul(cs_ps[:, :], ones_c56, mob[:, :],
                                 start=True, stop=True)
                nc.vector.tensor_reduce(sums[0:1, s:s + 1], cs_ps[:, :],
                                        mybir.AxisListType.X, ALU.add)

                # broadcast nm to [128, HW] for the channel loop:
                # flatten on the SWDGE ring, then PE-matmul broadcast
                # (ones[1,128]^T @ nm_flat chunk) with ACT copies to SBUF --
                # keeps the big-load HWDGE ring free.
                nmflat = slotp.tile([1, HW], F32, tag="nmflat")
                nc.scalar.dma_start(nmflat[0:1, :], nm[:, :])
                for k in range(7):
                    cols = slice(448 * k, 448 * (k + 1))
                    nmb_ps = psp.tile([128, 448], F32, tag="nmbps", bufs=2,
                                      name=f"nmbps_{s}_{k}")
                    nc.tensor.matmul(nmb_ps[:, :], ones_r128,
                                     nmflat[0:1, cols], start=True, stop=True)
                    nc.scalar.copy(nmb[s][:, cols], nmb_ps[:, :])
                # partition p holds channels (2p, 2p+1): replicate the mask
                # into the second 3136-wide half of the tile
                nc.scalar.copy(nmb[s][:, HW:CW], nmb[s][:, 0:HW])

            # ---- label outputs ----
            labq = smallp.tile([1, 4], F32, tag="labq")
            nc.vector.tensor_tensor(labq[:, :], sums[:, :], sc[0:1, 12:16],
                                    ALU.add)
            nc.vector.tensor_scalar(labq[:, :], labq[:, :], 0.0, None,
                                    ALU.is_gt)
            nc.gpsimd.dma_start(lout.ap(), labq[:, :])

            # ---- big stream: perturbed = input + noise * nm ----
            for j in range(2):
                for ct in range(CT):
                    it = its[j * CT + ct]
                    for slot in (2 * j, 2 * j + 1):
                        nz = bnzp.tile([128, CW], BF16, tag="nz")
                        nc.sync.dma_start(nz[:, :], noz.ap()[slot, ct])
                        nc.vector.tensor_tensor(nz[:, :], nz[:, :],
                                                nmb[slot][:, :], ALU.mult)
                        nc.vector.tensor_tensor(nz[:, :], nz[:, :],
                                                it[:, :], ALU.add)
                        nc.scalar.dma_start(pert.ap()[slot, ct], nz[:, :])

    nc.compile()
    return nc


def _get_program():
    if "nc" not in _PROGRAM_CACHE:
        _PROGRAM_CACHE["nc"] = _build_program()
    return _PROGRAM_CACHE["nc"]


def _make_in_maps(input, mask, labels, noise, angles, chance, scale_x, scale_y):
    in_maps = []
    for i in range(N_CORES):
        r0, r1 = 2 * i, 2 * i + 1
        slots = [r0, r0 + B, r1, r1 + B]          # duplicated-batch indices
        rows = [r0 % B, (r0 + B) % B, r1 % B, (r1 + B) % B]
        scl_row = np.concatenate([
            scale_x[slots].astype(np.float32),
            scale_y[slots].astype(np.float32),
            chance[slots].astype(np.float32),
            labels[rows, 0].astype(np.float32),
        ]).reshape(1, 16)
        in_maps.append({
            "inp": np.ascontiguousarray(
                input[r0:r1 + 1].reshape(2, CT, 128, CW)
                .astype(ml_dtypes.bfloat16)),
            "noz": np.ascontiguousarray(
                noise[slots].reshape(4, CT, 128, CW)
                .astype(ml_dtypes.bfloat16)),
            "msk": np.ascontiguousarray(mask[r0:r1 + 1, 0]),
            "ang": np.ascontiguousarray(angles[slots]),
            "scl": np.ascontiguousarray(scl_row),
        })
    return in_maps


def kernel(input, mask, labels, noise, angles, chance, scale_x, scale_y):
    input = np.asarray(input, np.float32)
    mask = np.asarray(mask, np.float32)
    labels = np.asarray(labels, np.float32)
    noise = np.asarray(noise, np.float32)
    angles = np.asarray(angles, np.float32)
    chance = np.asarray(chance, np.float32)
    scale_x = np.asarray(scale_x)
    scale_y = np.asarray(scale_y)

    nc = _get_program()
    in_maps = _make_in_maps(input, mask, labels, noise, angles, chance,
                            scale_x, scale_y)
    res = run_bass_kernel_spmd(nc, in_maps, core_ids=list(range(N_CORES)),
                               trace=TRACE)
    if TRACE and res.exec_time_ns is not None:
        print(f"HW exec time: {res.exec_time_ns} ns")
        _PROGRAM_CACHE["last_result"] = res

    perturbed = np.empty((2 * B, C, H, W), np.float32)
    mask_out = np.empty((2 * B, 1, H, W), np.int32)
    lab_out = np.empty((2 * B,), np.int32)
    for i in range(N_CORES):
        r0, r1 = 2 * i, 2 * i + 1
        slots = [r0, r0 + B, r1, r1 + B]
        r = res.results[i]
        p = r["pert"].astype(np.float32).reshape(4, C, H, W)
        m = r["mout"]
        l = r["lout"].reshape(4)
        for s, sg in enumerate(slots):
            perturbed[sg] = p[s]
            mask_out[sg, 0] = m[s].astype(np.int32)
            lab_out[sg] = np.int32(l[s])
    return perturbed, mask_out, lab_out


# revision 11
# speedup vs baseline: 1.2107x; 1.2107x over previous
"""Trainium2 Bass kernel for nn_AnomalyGenerator (8-core data parallel).

The reference duplicates the batch (16 -> 32 samples), builds per-sample
Perlin threshold masks (64x64 canvas, bilinear-resized to 56x56), and
computes perturbed = input + noise * noise_mask over [32,768,56,56].

Key algebraic reduction: the Perlin gather + fade interpolation + bilinear
resize is linear in the per-sample gradient tables, so
    pn56^T = RBxT[ey]^T @ (gx^T @ LAxT[ex]) + RByT[ey]^T @ (gy^T @ LAyT[ex])
with 6-entry compile-time tables (scales are 2^e, e in 0..5), gx=cos(angles),
gy=sin(angles).  Four [33,*] matmuls per sample on the PE.

Sharding: core i owns input rows {2i, 2i+1} and duplicated samples
(slots) [2i, 2i+16, 2i+1, 2i+17] -- input rows are read once per core.
"""

import math
import ml_dtypes
import numpy as np

import concourse.bass as bass
import concourse.bacc as bacc
import concourse.mybir as mybir
import concourse.tile as tile
from concourse.bass_utils import run_bass_kernel_spmd

F32 = mybir.dt.float32
BF16 = mybir.dt.bfloat16
ALU = mybir.AluOpType
ACT = mybir.ActivationFunctionType

N_CORES = 8
B = 16            # original batch
C = 768
H = W = 56
HW = H * W        # 3136
CT = 3            # channel tiles of 256 (2 channels per partition, bf16)
CW = 2 * HW       # free width of a big tile: 2 channel rows of 3136
SQRT2 = 1.4142135623730951
BIG = 1.0e9

TRACE = False     # test.py sets True to collect HW exec time
_PROGRAM_CACHE = {}


def _fade(t):
    return t * t * t * (t * (t * 6.0 - 15.0) + 10.0)


def _build_tables():
    """ctab [33, 6*4*56] and cmisc [56, 128] compile-time constants."""
    # bilinear resize matrix L [56,64], align_corners=False
    L = np.zeros((56, 64), np.float64)
    sy = np.clip((np.arange(56) + 0.5) * (64.0 / 56.0) - 0.5, 0.0, 63.0)
    y0 = np.floor(sy).astype(int)
    y1 = np.minimum(y0 + 1, 63)
    wy = sy - y0
    for h in range(56):
        L[h, y0[h]] += 1.0 - wy[h]
        L[h, y1[h]] += wy[h]

    ctab = np.zeros((33, 6 * 4 * 56), np.float64)
    for e in range(6):
        s = 2 ** e
        i = np.arange(64)
        ix = i * s
        c0 = ix // 64
        fu = (ix % 64).astype(np.float64) / 64.0
        tu = _fade(fu)
        U0 = np.zeros((64, 33))
        U1 = np.zeros((64, 33))
        U0[np.arange(64), c0] = 1.0
        U1[np.arange(64), c0 + 1] = 1.0
        Ax = (((1 - tu) * fu)[:, None] * U0) + ((tu * (fu - 1))[:, None] * U1)
        Ay = ((1 - tu)[:, None] * U0) + (tu[:, None] * U1)
        Bx = Ay                                   # v-factor for x-gradient
        By = Ax                                   # v-factor for y-gradient
        base = e * 224
        ctab[:, base + 0:base + 56] = (SQRT2 * (L @ Ax)).T     # LAxT
        ctab[:, base + 56:base + 112] = (SQRT2 * (L @ Ay)).T   # LAyT
        ctab[:, base + 112:base + 168] = (L @ Bx).T            # RBxT
        ctab[:, base + 168:base + 224] = (L @ By).T            # RByT

    cmisc = np.zeros((56, 256), np.float64)
    cmisc[:, 0:6] = np.arange(6)[None, :]         # iota row 0..5
    cmisc[:, 6:62] = np.eye(56)                   # identity for PE transpose
    cmisc[:, 62:190] = 1.0                        # ones block (128 wide)
    cmisc[:, 190] = math.pi                       # pi (activation bias)
    return ctab.astype(np.float32), cmisc.astype(np.float32)


def _build_program():
    """Build + compile the per-core Bass program (same NEFF on all 8 cores)."""
    ctab_np, cmisc_np = _build_tables()

    nc = bacc.Bacc("TRN2", target_bir_lowering=False, debug=False,
                   num_devices=N_CORES)

    inp = nc.dram_tensor("inp", [2, CT, 128, CW], BF16, kind="ExternalInput")
    noz = nc.dram_tensor("noz", [4, CT, 128, CW], BF16, kind="ExternalInput")
    msk = nc.dram_tensor("msk", [2, H, W], F32, kind="ExternalInput")
    ang = nc.dram_tensor("ang", [4, 33, 33], F32, kind="ExternalInput")
    scl = nc.dram_tensor("scl", [1, 16], F32, kind="ExternalInput")

    pert = nc.dram_tensor("pert", [4, CT, 128, CW], BF16, kind="ExternalOutput")
    mout = nc.dram_tensor("mout", [4, H, W], F32, kind="ExternalOutput")
    lout = nc.dram_tensor("lout", [1, 4], F32, kind="ExternalOutput")

    ctab_h = nc.inline_tensor(ctab_np, "ctab")
    cmisc_h = nc.inline_tensor(cmisc_np, "cmisc")

    with tile.TileContext(nc) as tc:
        with (
            tc.tile_pool(name="const", bufs=1) as constp,
            tc.tile_pool(name="small", bufs=1) as smallp,
            tc.tile_pool(name="sel", bufs=2) as selp,
            tc.tile_pool(name="slot", bufs=2) as slotp,
            tc.tile_pool(name="nmb", bufs=1) as nmbp,
            tc.tile_pool(name="bigin", bufs=6) as binp,
            tc.tile_pool(name="bignz", bufs=4) as bnzp,
            tc.tile_pool(name="ps", bufs=1, space="PSUM") as psp,
        ):
            # ---- constants into SBUF ----
            tabs = constp.tile([33, 6 * 4 * 56], F32, tag="tabs")
            misc = constp.tile([56, 256], F32, tag="misc")
            nc.sync.dma_start(tabs[:, :], ctab_h.ap())
            nc.sync.dma_start(misc[:, :], cmisc_h.ap())
            iota33 = misc[0:33, 0:6]
            eye56 = misc[0:56, 6:62]
            ones_r33 = misc[0:1, 62:95]
            ones_r56 = misc[0:1, 62:118]
            ones_r128 = misc[0:1, 62:190]
            ones_c56 = misc[0:56, 62:63]
            pi33 = misc[0:33, 190:191]

            ones16 = smallp.tile([1, 128], BF16, tag="ones16")
            nc.vector.tensor_copy(ones16[:, :], misc[0:1, 62:190])

            # ---- small per-core inputs ----
            sc = smallp.tile([1, 16], F32, tag="sc")
            nc.sync.dma_start(sc[:, :], scl.ap())
            gang = smallp.tile([33, 132], F32, tag="gang")
            for s in range(4):
                nc.sync.dma_start(gang[:, 33 * s:33 * (s + 1)], ang.ap()[s])
            msk_sb = smallp.tile([56, 112], F32, tag="msk")
            for j in range(2):
                nc.sync.dma_start(msk_sb[:, 56 * j:56 * (j + 1)], msk.ap()[j])

            # input loads first on the SWDGE (gpsimd) ring -- nothing with a
            # compute-wait may precede them there, so they stream from t=0
            its = []
            for j in range(2):
                for ct in range(CT):
                    it = binp.tile([128, CW], BF16, tag="in",
                                   name=f"it_{j}_{ct}")
                    nc.gpsimd.dma_start(it[:, :], inp.ap()[j, ct])
                    its.append(it)

            # ---- gradients: gx = cos(angles), gy = sin(angles)  [33,132] ----
            gy = smallp.tile([33, 132], F32, tag="gy")
            # sin(x) = sin(pi - x), arg in [-pi, pi]
            nc.scalar.activation(gy[:, :], gang[:, :], ACT.Sin,
                                 bias=pi33, scale=-1.0)
            gx = smallp.tile([33, 132], F32, tag="gx")
            u = smallp.tile([33, 132], F32, tag="u")
            nc.scalar.activation(u[:, :], gang[:, :], ACT.Sin, scale=0.5)
            nc.scalar.activation(u[:, :], u[:, :], ACT.Square)
            # cos(x) = 1 - 2 sin^2(x/2)
            nc.vector.tensor_scalar(gx[:, :], u[:, :], -2.0, 1.0,
                                    ALU.mult, ALU.add)

            # ---- broadcast per-slot scalars to 33 partitions ----
            prep_ps = psp.tile([33, 16], F32, tag="aux")
            nc.tensor.matmul(prep_ps[:, :], ones_r33, sc[:, :],
                             start=True, stop=True)
            scb33 = smallp.tile([33, 16], F32, tag="scb33")
            nc.vector.tensor_copy(scb33[:, :], prep_ps[:, :])

            # one-hot scale selectors wxb/wyb [33, 24] (6 per slot)
            wxb = smallp.tile([33, 24], F32, tag="wxb")
            wyb = smallp.tile([33, 24], F32, tag="wyb")
            for s in range(4):
                nc.vector.tensor_scalar(wxb[:, 6 * s:6 * (s + 1)], iota33,
                                        scb33[:, s:s + 1], None, ALU.is_equal)
                nc.vector.tensor_scalar(wyb[:, 6 * s:6 * (s + 1)], iota33,
                                        scb33[:, 4 + s:5 + s], None,
                                        ALU.is_equal)

            # ---- per-slot threshold + (1-label) rows, broadcast to 56 ----
            sc8 = smallp.tile([1, 8], F32, tag="sc8")
            # T_s = 0.5 + BIG*(chance>0.5)
            nc.vector.tensor_scalar(sc8[:, 0:4], sc[0:1, 8:12], 0.5, None,
                                    ALU.is_gt)
            nc.vector.tensor_scalar(sc8[:, 0:4], sc8[:, 0:4], BIG, 0.5,
                                    ALU.mult, ALU.add)
            # (1 - label)
            nc.vector.tensor_scalar(sc8[:, 4:8], sc[0:1, 12:16], -1.0, 1.0,
                                    ALU.mult, ALU.add)
            prep2_ps = psp.tile([56, 8], F32, tag="aux")
            nc.tensor.matmul(prep2_ps[:, :], ones_r56, sc8[:, :],
                             start=True, stop=True)
            thb56 = smallp.tile([56, 8], F32, tag="thb56")
            nc.vector.tensor_copy(thb56[:, :], prep2_ps[:, :])

            # (1 - mask) for the two input rows
            um_sb = smallp.tile([56, 112], F32, tag="um")
            nc.vector.tensor_scalar(um_sb[:, :], msk_sb[:, :], -1.0, 1.0,
                                    ALU.mult, ALU.add)

            sums = smallp.tile([1, 4], F32, tag="sums")
            nmb = [nmbp.tile([128, CW], BF16, tag=f"nmb{s}", name=f"nmb{s}")
                   for s in range(4)]

            # ---- per-slot perlin mask chain ----
            for s in range(4):
                j = s // 2  # input row index for this slot
                selx = selp.tile([33, 224], F32, tag="sel")
                # select tables for this slot's (ex, ey) via one-hot weights
                nc.vector.tensor_scalar(
                    selx[:, 0:112], tabs[:, 0:112],
                    wxb[:, 6 * s:6 * s + 1], None, ALU.mult)
                nc.vector.tensor_scalar(
                    selx[:, 112:224], tabs[:, 112:224],
                    wyb[:, 6 * s:6 * s + 1], None, ALU.mult)
                for e in range(1, 6):
                    nc.vector.scalar_tensor_tensor(
                        selx[:, 0:112], tabs[:, 224 * e:224 * e + 112],
                        wxb[:, 6 * s + e:6 * s + e + 1], selx[:, 0:112],
                        ALU.mult, ALU.add)
                    nc.vector.scalar_tensor_tensor(
                        selx[:, 112:224], tabs[:, 224 * e + 112:224 * e + 224],
                        wyb[:, 6 * s + e:6 * s + e + 1], selx[:, 112:224],
                        ALU.mult, ALU.add)

                gx_s = gx[:, 33 * s:33 * (s + 1)]
                gy_s = gy[:, 33 * s:33 * (s + 1)]
                p1_ps = psp.tile([33, 112], F32, tag="p1")
                nc.tensor.matmul(p1_ps[:, 0:56], gx_s, selx[:, 0:56],
                                 start=True, stop=True)
                nc.tensor.matmul(p1_ps[:, 56:112], gy_s, selx[:, 56:112],
                                 start=True, stop=True)
                p1s = slotp.tile([33, 112], F32, tag="p1s")
                nc.scalar.copy(p1s[:, :], p1_ps[:, :])

                pnT_ps = psp.tile([56, 56], F32, tag="pnT")
                nc.tensor.matmul(pnT_ps[:, :], selx[:, 112:168], p1s[:, 0:56],
                                 start=True, stop=False)
                nc.tensor.matmul(pnT_ps[:, :], selx[:, 168:224],
                                 p1s[:, 56:112], start=False, stop=True)
                pnT_sb = slotp.tile([56, 56], F32, tag="pnT_sb")
                nc.scalar.copy(pnT_sb[:, :], pnT_ps[:, :])

                pn_ps = psp.tile([56, 56], F32, tag="pn")
                nc.tensor.transpose(pn_ps[:, :], pnT_sb[:, :], eye56)

                thr = slotp.tile([56, 56], F32, tag="thr")
                nc.vector.tensor_scalar(thr[:, :], pn_ps[:, :],
                                        thb56[:, s:s + 1], None, ALU.is_gt)
                nm = slotp.tile([56, 56], BF16, tag="nm")
                nc.vector.scalar_tensor_tensor(
                    nm[:, :], thr[:, :], thb56[:, 4 + s:5 + s],
                    um_sb[:, 56 * j:56 * (j + 1)], ALU.mult, ALU.mult)

                mob = slotp.tile([56, 56], F32, tag="mob")
                nc.vector.scalar_tensor_tensor(
                    mob[:, :], thr[:, :], thb56[:, 4 + s:5 + s],
                    msk_sb[:, 56 * j:56 * (j + 1)], ALU.mult, ALU.add)
                nc.vector.tensor_scalar(mob[:, :], mob[:, :], 0.0, None,
                                        ALU.is_gt)
                nc.gpsimd.dma_start(mout.ap()[s], mob[:, :])

                cs_ps = psp.tile([1, 56], F32, tag="aux")
                nc.tensor.matmul(cs_ps[:, :], ones_c56, mob[:, :],
                                 start=True, stop=True)
                nc.vector.tensor_reduce(sums[0:1, s:s + 1], cs_ps[:, :],
                                        mybir.AxisListType.X, ALU.add)

                # broadcast nm to [128, HW] for the channel loop:
                # flatten on the SWDGE ring, then PE-matmul broadcast
                # (ones[1,128]^T @ nm_flat chunk) with ACT copies to SBUF --
                # keeps the big-load HWDGE ring free.
                nmflat = slotp.tile([1, HW], BF16, tag="nmflat")
                nc.scalar.dma_start(nmflat[0:1, :], nm[:, :])
                for k in range(7):
                    cols = slice(448 * k, 448 * (k + 1))
                    cols2 = slice(HW + 448 * k, HW + 448 * (k + 1))
                    nmb_ps = psp.tile([128, 448], F32, tag="nmbps", bufs=4,
                                      name=f"nmbps_{s}_{k}")
                    nc.tensor.matmul(nmb_ps[:, :], ones16[:, :],
                                     nmflat[0:1, cols], start=True, stop=True)
                    # partition p holds channels (2p, 2p+1): write the mask
                    # into both 3136-wide halves of the tile
                    nc.scalar.copy(nmb[s][:, cols], nmb_ps[:, :])
                    nc.scalar.copy(nmb[s][:, cols2], nmb_ps[:, :])

            # ---- label outputs ----
            labq = smallp.tile([1, 4], F32, tag="labq")
            nc.vector.tensor_tensor(labq[:, :], sums[:, :], sc[0:1, 12:16],
                                    ALU.add)
            nc.vector.tensor_scalar(labq[:, :], labq[:, :], 0.0, None,
                                    ALU.is_gt)
            nc.gpsimd.dma_start(lout.ap(), labq[:, :])

            # ---- big stream: perturbed = input + noise * nm ----
            for j in range(2):
                for ct in range(CT):
                    it = its[j * CT + ct]
                    for slot in (2 * j, 2 * j + 1):
                        nz = bnzp.tile([128, CW], BF16, tag="nz")
                        nc.sync.dma_start(nz[:, :], noz.ap()[slot, ct])
                        nc.vector.tensor_tensor(nz[:, :], nz[:, :],
                                                nmb[slot][:, :], ALU.mult)
                        nc.vector.tensor_tensor(nz[:, :], nz[:, :],
                                                it[:, :], ALU.add)
                        nc.scalar.dma_start(pert.ap()[slot, ct], nz[:, :])

    nc.compile()
    return nc


def _get_program():
    if "nc" not in _PROGRAM_CACHE:
        _PROGRAM_CACHE["nc"] = _build_program()
    return _PROGRAM_CACHE["nc"]


def _make_in_maps(input, mask, labels, noise, angles, chance, scale_x, scale_y):
    in_maps = []
    for i in range(N_CORES):
        r0, r1 = 2 * i, 2 * i + 1
        slots = [r0, r0 + B, r1, r1 + B]          # duplicated-batch indices
        rows = [r0 % B, (r0 + B) % B, r1 % B, (r1 + B) % B]
        scl_row = np.concatenate([
            scale_x[slots].astype(np.float32),
            scale_y[slots].astype(np.float32),
            chance[slots].astype(np.float32),
            labels[rows, 0].astype(np.float32),
        ]).reshape(1, 16)
        in_maps.append({
            "inp": np.ascontiguousarray(
                input[r0:r1 + 1].reshape(2, CT, 128, CW)
                .astype(ml_dtypes.bfloat16)),
            "noz": np.ascontiguousarray(
                noise[slots].reshape(4, CT, 128, CW)
                .astype(ml_dtypes.bfloat16)),
            "msk": np.ascontiguousarray(mask[r0:r1 + 1, 0]),
            "ang": np.ascontiguousarray(angles[slots]),
            "scl": np.ascontiguousarray(scl_row),
        })
    return in_maps


def kernel(input, mask, labels, noise, angles, chance, scale_x, scale_y):
    input = np.asarray(input, np.float32)
    mask = np.asarray(mask, np.float32)
    labels = np.asarray(labels, np.float32)
    noise = np.asarray(noise, np.float32)
    angles = np.asarray(angles, np.float32)
    chance = np.asarray(chance, np.float32)
    scale_x = np.asarray(scale_x)
    scale_y = np.asarray(scale_y)

    nc = _get_program()
    in_maps = _make_in_maps(input, mask, labels, noise, angles, chance,
                            scale_x, scale_y)
    res = run_bass_kernel_spmd(nc, in_maps, core_ids=list(range(N_CORES)),
                               trace=TRACE)
    if TRACE and res.exec_time_ns is not None:
        print(f"HW exec time: {res.exec_time_ns} ns")
        _PROGRAM_CACHE["last_result"] = res

    perturbed = np.empty((2 * B, C, H, W), np.float32)
    mask_out = np.empty((2 * B, 1, H, W), np.int32)
    lab_out = np.empty((2 * B,), np.int32)
    for i in range(N_CORES):
        r0, r1 = 2 * i, 2 * i + 1
        slots = [r0, r0 + B, r1, r1 + B]
        r = res.results[i]
        p = r["pert"].astype(np.float32).reshape(4, C, H, W)
        m = r["mout"]
        l = r["lout"].reshape(4)
        for s, sg in enumerate(slots):
            perturbed[sg] = p[s]
            mask_out[sg, 0] = m[s].astype(np.int32)
            lab_out[sg] = np.int32(l[s])
    return perturbed, mask_out, lab_out
